# revision 25
# baseline (speedup 1.0000x reference)
"""Trainium2 Bass kernel for nn_Attention_58153857187952.

Dense transformer block: QKV -> masked softmax attention (with a global-max
mask bias) -> concat proj -> post-LN residual -> FFN(gelu) -> post-LN.

Sharding: batch data-parallel, 1 batch element per core (B=8, 8 cores).

Math: the reference computes
    attn = softmax(qk + (1-m)*(-gmax)) * m,   gmax = max(qk) over ALL b,h,i,j
Softmax rows decompose:
    out_ij = p_ij * keep_j / (D1_i + e^{-gmax} * D2_i),  p = exp(qk)
with D1 = sum_keep p, D2 = sum_masked p.  Scores are bounded (|qk| < ~8) so
exp needs no row-max subtraction.  e^{-gmax} enters only as a ~0.3%
denominator correction, so a per-core sampled max is numerically
indistinguishable from the global max -> no collective needed.

Perf structure (v3):
  * QKV + concat projections in fp8 (e4m3) DoubleRow (2 k-rows per pass);
    weights and src^T packed [128, 2, C] host-side.
  * Score matmuls row-packed per head pair (K=64 at rows 0-63 / 64-127),
    measured concurrent (delta-start ~4ns).
  * V carries [keep, 1-keep] stat columns (66-wide lhsT) so D1/D2 fall out
    of the PV matmul with zero extra streaming.
  * Per-head-pair denominator/scale chains pipelined under the attention
    loop; e^{-gmax} sampled from the first 4 pairs so the chain starts
    early.  Only the last pair's chain is exposed.
  * exp/gelu/psum-copies on [128,1024] tiles; layernorm via bn_stats;
    gains/adds on GpSimd (otherwise idle); b_concat folded into src and
    b_ffn2 into the FFN2 accumulator host/copy-side.
  * x1^T via per-block DMA xbar transposes, pipelined with LN1; FFN bf16.
"""

import os
import sys

import numpy as np

sys.path.insert(0, "/opt/trn_rl_repo")

from contextlib import ExitStack

import concourse.bass as bass
import concourse.tile as tile
from concourse import bacc
from concourse import mybir
from concourse.bass import ts

B, N, D, H = 8, 1024, 1024, 16
HD = D // H
SCALE = HD ** -0.5
EPS = 1e-5
P = 128
NT = N // P          # 8 token tiles
DT = D // P          # 8 feature tiles
KPR = D // 256       # 4 packed k-pair groups
C4 = 4 * D           # 4096
NCORES = 8
HW = HD + 2          # 66: head dims + [keep, kinv] stat columns
EXPC = 3.0           # exp shift: p' = exp(qk*SCALE - C) keeps fp8 < 240
EGC = float(np.exp(-EXPC))

F32 = mybir.dt.float32
BF16 = mybir.dt.bfloat16
F8 = mybir.dt.float8e4
AX = mybir.AxisListType.X
ALU = mybir.AluOpType
ACTF = mybir.ActivationFunctionType
DR = mybir.MatmulPerfMode.DoubleRow


def _bc(ap, parts):
    """Partition-broadcast a [1, ...] DRAM AP across `parts` partitions."""
    return bass.AP(tensor=ap.tensor, offset=ap.offset, ap=[[0, parts]] + list(ap.ap[1:]))


def build(nc):
    # ---------------- I/O ----------------
    src = nc.declare_dram_parameter("src", [N, D], F32, isOutput=False)  # src + b_concat
    srcT8 = nc.declare_dram_parameter("srcT8", [KPR * P, 2 * N], F8, isOutput=False)
    wq8 = nc.declare_dram_parameter("wq8", [KPR * P, 2 * D], F8, isOutput=False)
    wk8 = nc.declare_dram_parameter("wk8", [KPR * P, 2 * D], F8, isOutput=False)
    wv8 = nc.declare_dram_parameter("wv8", [KPR * P, 2 * D], F8, isOutput=False)
    wc8 = nc.declare_dram_parameter("wc8", [KPR * P, 2 * D], F8, isOutput=False)
    w1 = nc.declare_dram_parameter("w1", [D, C4], BF16, isOutput=False)
    w2 = nc.declare_dram_parameter("w2", [C4, D], BF16, isOutput=False)
    keep_row = nc.declare_dram_parameter("keep_row", [1, N], F32, isOutput=False)
    keep_col = nc.declare_dram_parameter("keep_col", [P, NT], F32, isOutput=False)
    sgn_row = nc.declare_dram_parameter("sgn_row", [P, 2 * H], BF16, isOutput=False)
    ofs_row = nc.declare_dram_parameter("ofs_row", [P, 2 * H], BF16, isOutput=False)
    b1_col = nc.declare_dram_parameter("b1_col", [P, C4 // P], F32, isOutput=False)
    b2_row = nc.declare_dram_parameter("b2_row", [1, D], F32, isOutput=False)
    g1_row = nc.declare_dram_parameter("g1_row", [1, D], F32, isOutput=False)
    bg1_row = nc.declare_dram_parameter("bg1_row", [1, D], F32, isOutput=False)
    g2_row = nc.declare_dram_parameter("g2_row", [1, D], F32, isOutput=False)
    bg2_row = nc.declare_dram_parameter("bg2_row", [1, D], F32, isOutput=False)
    out = nc.declare_dram_parameter("out", [N, D], F32, isOutput=True)

    # internal DRAM scratch
    dstat_dram = nc.dram_tensor("dstat_dram", [2 * H, N], F32)  # rows h: D1, 16+h: D2
    gcol_dram = nc.dram_tensor("gcol_dram", [P, 1], F32)
    eg_dram = nc.dram_tensor("eg_dram", [1, 1], F32)
    s16_dram = nc.dram_tensor("s16_dram", [H, N], F32)
    x1bd_dram = nc.dram_tensor("x1bd_dram", [N, D], BF16)

    def ln_natural(pool, xin, yout, tagp, g_b=None, bg_b=None):
        """Layernorm along free dim of a [P, D] tile into caller tile yout.

        Without g_b/bg_b, writes the normalized value directly (gains folded
        into downstream weights host-side)."""
        stats = pool.tile([P, 2, 6], F32, name=tagp + "st", tag=tagp + "st")
        xr = xin.rearrange("p (a b) -> p a b", b=512)
        nc.vector.bn_stats(out=stats[:, 0, :], in_=xr[:, 0, :])
        nc.vector.bn_stats(out=stats[:, 1, :], in_=xr[:, 1, :])
        mv = pool.tile([P, 2], F32, name=tagp + "mv", tag=tagp + "mv")
        nc.vector.bn_aggr(out=mv, in_=stats)
        std = pool.tile([P, 1], F32, name=tagp + "s5", tag=tagp + "s5")
        nc.scalar.activation(std, mv[:, 1:2], ACTF.Sqrt, bias=eps_t)
        rstd = pool.tile([P, 1], F32, name=tagp + "s6", tag=tagp + "s6")
        nc.vector.reciprocal(rstd, std)
        nc.vector.tensor_scalar(out=yout, in0=xin, scalar1=mv[:, 0:1],
                                scalar2=rstd, op0=ALU.subtract, op1=ALU.mult)
        if g_b is not None:
            nc.vector.tensor_mul(yout, yout, g_b)
            nc.vector.tensor_add(yout, yout, bg_b)

    with ExitStack() as ctx:
        tc = ctx.enter_context(tile.TileContext(nc))
        sing = ctx.enter_context(tc.tile_pool(name="sing", bufs=1))
        psS = ctx.enter_context(tc.tile_pool(name="psS", bufs=3, space="PSUM"))
        psU = ctx.enter_context(tc.tile_pool(name="psU", bufs=2, space="PSUM"))
        x1T_pool = ctx.enter_context(tc.tile_pool(name="x1T", bufs=1))
        x1_pool = ctx.enter_context(tc.tile_pool(name="x1p", bufs=1))

        keepc = sing.tile([P, NT], F32)
        nc.sync.dma_start(out=keepc, in_=keep_col[:])
        b1c = sing.tile([P, C4 // P], F32)
        nc.sync.dma_start(out=b1c, in_=b1_col[:])
        sgn = sing.tile([P, 2 * H], BF16)
        nc.sync.dma_start(out=sgn, in_=sgn_row[:])
        ofs = sing.tile([P, 2 * H], BF16)
        nc.sync.dma_start(out=ofs, in_=ofs_row[:])
        pmax_slots = sing.tile([P, 4], F32)
        eps_t = sing.tile([P, 1], F32)
        nc.vector.memset(eps_t, EPS)
        negc_t = sing.tile([P, 1], F32)
        nc.vector.memset(negc_t, -EXPC)

        x1T = [x1T_pool.tile([P, N], BF16, name=f"x1T{t}") for t in range(DT)]
        x1n = [x1_pool.tile([P, D], BF16, name=f"x1n{t}") for t in range(NT)]
        x1g2 = [x1_pool.tile([P, D], BF16, name=f"x1g2_{t}") for t in range(NT)]

        with tc.tile_pool(name="attp", bufs=1) as attp, \
             tc.tile_pool(name="wc8p", bufs=1) as wc8p:
          attnT = [attp.tile([P, N], BF16, name=f"attnT{t}") for t in range(DT)]
          attnT8 = [attp.tile([P, 2, N], F8, name=f"attnT8_{t}") for t in range(KPR)]
          wc8t = [wc8p.tile([P, 2, D], F8, name=f"wc8_{k}") for k in range(KPR)]
          for k in range(KPR):
              nc.sync.dma_start(out=wc8t[k], in_=wc8[ts(k, P), :])

          with tc.tile_pool(name="qkvp", bufs=1) as qkvp:
            QT = [qkvp.tile([P, N], BF16, name=f"qt{t}") for t in range(DT)]
            KT = [qkvp.tile([P, N], BF16, name=f"kt{t}") for t in range(DT)]
            VA8 = [qkvp.tile([P, 2, H * HW], F8, name=f"va{t}")
                   for t in range(NT // 2)]

            # ---------- phase 0/1: load packed operands, QKV projections ----
            with tc.tile_pool(name="w8p", bufs=1) as w8p:
                st8 = [w8p.tile([P, 2, N], F8, name=f"st8_{k}") for k in range(KPR)]
                wq8t = [w8p.tile([P, 2, D], F8, name=f"wq8_{k}") for k in range(KPR)]
                wk8t = [w8p.tile([P, 2, D], F8, name=f"wk8_{k}") for k in range(KPR)]
                wv8t = [w8p.tile([P, 2, D], F8, name=f"wv8_{k}") for k in range(KPR)]
                for k in range(KPR):
                    nc.sync.dma_start(out=st8[k], in_=srcT8[ts(k, P), :])
                    nc.sync.dma_start(out=wq8t[k], in_=wq8[ts(k, P), :])

                # Q^T then K^T: [dq, i] tiles; two heads per tile t
                def qk_proj(w8, dstT, t):
                    pt = psS.tile([P, 1024], F32, tag="mm", name="pqk")
                    for nb in range(2):
                        for k in range(KPR):
                            nc.tensor.matmul(
                                pt[:, ts(nb, 512)],
                                w8[k][:, :, ts(t, P)],
                                st8[k][:, :, ts(nb, 512)],
                                start=(k == 0), stop=(k == KPR - 1),
                                perf_mode=DR)
                    nc.scalar.copy(out=dstT[t], in_=pt)

                for t in range(DT):
                    qk_proj(wq8t, QT, t)
                for k in range(KPR):
                    nc.sync.dma_start(out=wk8t[k], in_=wk8[ts(k, P), :])
                for t in range(DT):
                    qk_proj(wk8t, KT, t)
                for k in range(KPR):
                    nc.sync.dma_start(out=wv8t[k], in_=wv8[ts(k, P), :])
                # V natural [token, dv], keep-zeroed rows + stat columns,
                # packed 2 token-tiles deep for DoubleRow PV
                for it in range(NT):
                    dst = VA8[it // 2][:, it % 2, :].rearrange(
                        "p (h c) -> p h c", c=HW)
                    for nb in range(2):
                        vps = psU.tile([P, 512], F32, tag="u", name="pv")
                        for k in range(KPR):
                            nc.tensor.matmul(
                                vps,
                                st8[k][:, :, ts(it, P)],
                                wv8t[k][:, :, ts(nb, 512)],
                                start=(k == 0), stop=(k == KPR - 1),
                                perf_mode=DR)
                        nc.vector.tensor_scalar(
                            out=dst[:, nb * 8:(nb + 1) * 8, 0:HD],
                            in0=vps.rearrange("p (h c) -> p h c", c=HD),
                            scalar1=keepc[:, it:it + 1], scalar2=None,
                            op0=ALU.mult)
                    # stat cols [keep, kinv] via sgn*keep + ofs
                    kk = qkvp.tile([P, 2 * H], BF16, name="kkt", tag="kkt")
                    nc.vector.tensor_scalar(
                        out=kk, in0=sgn, scalar1=keepc[:, it:it + 1],
                        scalar2=None, op0=ALU.mult)
                    nc.vector.tensor_tensor(
                        out=dst[:, :, HD:HW],
                        in0=kk.rearrange("p (h c) -> p h c", c=2),
                        in1=ofs.rearrange("p (h c) -> p h c", c=2),
                        op=ALU.add)

            # ---------- phase 2+3: attention with pipelined stat chains ----
            with tc.tile_pool(name="ptp", bufs=12) as ptp, \
                 tc.tile_pool(name="st2p", bufs=3) as st2p, \
                 tc.tile_pool(name="ep", bufs=1) as ep, \
                 tc.tile_pool(name="srp", bufs=2) as srp:

                keep128 = ep.tile([P, N], F32)
                nc.sync.dma_start(out=keep128, in_=_bc(keep_row[:], P))
                egp = ep.tile([P, 1], F32)

                def eg_chain():
                    # local sampled max of exp scores -> e^{-gmax} broadcast
                    gmax128 = ep.tile([P, 1], F32)
                    nc.vector.reduce_max(out=gmax128, in_=pmax_slots, axis=AX)
                    nc.sync.dma_start(out=gcol_dram[:], in_=gmax128)
                    grow = ep.tile([1, P], F32)
                    nc.sync.dma_start(out=grow, in_=bass.AP(
                        tensor=gcol_dram[:].tensor, offset=0, ap=[[0, 1], [1, P]]))
                    gmax1 = ep.tile([1, 1], F32)
                    nc.vector.reduce_max(out=gmax1, in_=grow, axis=AX)
                    eg1 = ep.tile([1, 1], F32)
                    nc.vector.reciprocal(eg1, gmax1)
                    nc.vector.tensor_scalar(out=eg1, in0=eg1, scalar1=EGC,
                                            scalar2=None, op0=ALU.mult)
                    nc.sync.dma_start(out=eg_dram[:], in_=eg1)
                    nc.sync.dma_start(out=egp, in_=_bc(eg_dram[:], P))

                def stat_chain(t):
                    # scale s(h,i) = keep_i / (D1 + eg*D2) for heads 2t, 2t+1,
                    # with D1/D2 broadcast-read straight from dstat rows
                    da = dstat_dram[:].tensor
                    b1 = srp.tile([P, N], F32, name="sb1", tag="sb1")
                    nc.scalar.dma_start(out=b1[0:HD, :], in_=bass.AP(
                        tensor=da, offset=2 * t * N, ap=[[0, HD], [1, N]]))
                    nc.scalar.dma_start(out=b1[HD:P, :], in_=bass.AP(
                        tensor=da, offset=(2 * t + 1) * N, ap=[[0, HD], [1, N]]))
                    b2t = srp.tile([P, N], F32, name="sb2", tag="sb2")
                    nc.scalar.dma_start(out=b2t[0:HD, :], in_=bass.AP(
                        tensor=da, offset=(H + 2 * t) * N, ap=[[0, HD], [1, N]]))
                    nc.scalar.dma_start(out=b2t[HD:P, :], in_=bass.AP(
                        tensor=da, offset=(H + 2 * t + 1) * N, ap=[[0, HD], [1, N]]))
                    ss = srp.tile([P, N], F32, name="ss", tag="ss")
                    nc.vector.tensor_scalar(out=ss, in0=b2t, scalar1=egp,
                                            scalar2=None, op0=ALU.mult)
                    nc.vector.tensor_add(ss, ss, b1)
                    nc.vector.reciprocal(ss, ss)
                    nc.vector.tensor_mul(ss, ss, keep128)
                    nc.vector.tensor_tensor(
                        out=attnT8[t // 2][:, t % 2, :],
                        in0=attnT[t], in1=ss, op=ALU.mult)

                for t in range(DT):  # head pair (2t, 2t+1)
                    for ih in range(2):
                        # PV accumulators for both heads, fed as exps land so
                        # the PE never idles a full HAM window
                        uts = [psU.tile([P, 512], F32, tag="u", name="ut")
                               for _ in range(2)]

                        def pv_step(jg, PA, PB):
                            for hh, Ptile in ((0, PA), (1, PB)):
                                p8 = Ptile.rearrange("p (s i) -> p s i", i=512)
                                nc.tensor.matmul(
                                    uts[hh][0:HW, :],
                                    VA8[jg][:, :, ts(2 * t + hh, HW)],
                                    p8,
                                    start=(jg == 0), stop=(jg == 3),
                                    perf_mode=DR)

                        ptiles = []
                        for jg in range(4):
                            SA = psS.tile([P, 1024], F32, tag="mm", name="sa")
                            SB = psS.tile([P, 1024], F32, tag="mm", name="sb")
                            for jj in range(2):
                                jt = 2 * jg + jj
                                nc.tensor.matmul(
                                    SA[:, ts(jj, 512)],
                                    KT[t][0:HD, ts(jt, P)],
                                    QT[t][0:HD, ts(ih, 512)],
                                    start=True, stop=True)
                                nc.tensor.matmul(
                                    SB[:, ts(jj, 512)],
                                    KT[t][HD:P, ts(jt, P)],
                                    QT[t][HD:P, ts(ih, 512)],
                                    start=True, stop=True)
                            PA = ptp.tile([P, 1024], F8, tag="pt", name="pa")
                            PB = ptp.tile([P, 1024], F8, tag="pt", name="pb")
                            nc.scalar.activation(PA, SA, ACTF.Exp, scale=SCALE,
                                                 bias=negc_t)
                            nc.scalar.activation(PB, SB, ACTF.Exp, scale=SCALE,
                                                 bias=negc_t)
                            if t < 2 and ih == 0 and jg == 1:
                                nc.vector.reduce_max(
                                    out=pmax_slots[:, 2 * t:2 * t + 1],
                                    in_=PA, axis=AX)
                                nc.vector.reduce_max(
                                    out=pmax_slots[:, 2 * t + 1:2 * t + 2],
                                    in_=PB, axis=AX)
                            ptiles.append((PA, PB))
                            if jg >= 1:
                                pv_step(jg - 1, *ptiles[jg - 1])
                        pv_step(3, *ptiles[3])
                        for hh in range(2):
                            h = 2 * t + hh
                            ut = uts[hh]
                            nc.vector.tensor_copy(
                                out=attnT[t][hh * HD:(hh + 1) * HD, ts(ih, 512)],
                                in_=ut[0:HD, :])
                            st2 = st2p.tile([2, 512], F32, name="st2", tag="st2")
                            nc.vector.tensor_copy(out=st2, in_=ut[HD:HW, :])
                            nc.sync.dma_start(
                                out=bass.AP(tensor=dstat_dram[:].tensor,
                                            offset=h * N + ih * 512,
                                            ap=[[H * N, 2], [1, 512]]),
                                in_=st2)
                    if t == 1:
                        eg_chain()
                    if t >= 1:
                        stat_chain(t - 1)
                stat_chain(DT - 1)

          # ---------- phase 4: concat proj + residual + LN1 + x1T ----------
          with tc.tile_pool(name="ctmp", bufs=3) as ctmp:
              for mt in range(NT):
                  x0ps = psS.tile([P, 1024], F32, tag="mm", name="x0ps")
                  for nb in range(2):
                      for k in range(KPR):
                          nc.tensor.matmul(
                              x0ps[:, ts(nb, 512)],
                              attnT8[k][:, :, ts(mt, P)],
                              wc8t[k][:, :, ts(nb, 512)],
                              start=(k == 0), stop=(k == KPR - 1),
                              perf_mode=DR)
                  sre = ctmp.tile([P, D], F32, name="sr", tag="sr")
                  nc.scalar.dma_start(out=sre, in_=src[ts(mt, P), :])
                  x0 = ctmp.tile([P, D], F32, name="x0", tag="x0")
                  nc.vector.scalar_tensor_tensor(
                      out=x0, in0=x0ps, scalar=0.0, in1=sre,
                      op0=ALU.add, op1=ALU.add)
                  # g1/bg1 folded into w1/b1 and the residual term below
                  ln_natural(ctmp, x0, x1n[mt], "c")
                  nc.sync.dma_start(out=x1bd_dram[ts(mt, P), :], in_=x1n[mt])
              for kt in range(DT):
                  nc.scalar.dma_start_transpose(
                      out=x1T[kt], in_=x1bd_dram[:, ts(kt, P)])

        # ---------- phase 5: FFN (two half-C4 passes) + pipelined LN2 ----
        with tc.tile_pool(name="ffn", bufs=1) as ffp, \
             tc.tile_pool(name="w1s", bufs=16) as w1s, \
             tc.tile_pool(name="w2s", bufs=24) as w2s, \
             tc.tile_pool(name="ftmp", bufs=3) as ftmp:
            g1_b = ffp.tile([P, D], F32)
            nc.sync.dma_start(out=g1_b, in_=_bc(g1_row[:], P))
            b2g_b = ffp.tile([P, D], F32)
            nc.sync.dma_start(out=b2g_b, in_=_bc(b2_row[:], P))
            g2_b = ffp.tile([P, D], F32)
            nc.sync.dma_start(out=g2_b, in_=_bc(g2_row[:], P))
            bg2_b = ffp.tile([P, D], F32)
            nc.sync.dma_start(out=bg2_b, in_=_bc(bg2_row[:], P))
            hT = [ffp.tile([P, N], BF16, name=f"hT{t}") for t in range(16)]
            xf2 = [ffp.tile([P, D], F32, name=f"xf{t}") for t in range(NT)]
            for ch2 in range(2):
                for chh in range(4):
                    w1t = [w1s.tile([P, 512], BF16, name=f"w1_{kt}", tag="w1")
                           for kt in range(DT)]
                    for kt in range(DT):
                        nc.sync.dma_start(
                            out=w1t[kt],
                            in_=w1[ts(kt, P),
                                   ch2 * 2048 + chh * 512:
                                   ch2 * 2048 + (chh + 1) * 512])
                    for mc in range(4):
                        pos = chh * 4 + mc
                        hps = psS.tile([P, 1024], F32, tag="mm", name="hps")
                        for nb in range(2):
                            for kt in range(DT):
                                nc.tensor.matmul(
                                    hps[:, ts(nb, 512)],
                                    w1t[kt][:, ts(mc, P)],
                                    x1T[kt][:, ts(nb, 512)],
                                    start=(kt == 0), stop=(kt == DT - 1))
                        nc.scalar.activation(
                            hT[pos], hps, ACTF.Gelu,
                            bias=b1c[:, ch2 * 16 + pos: ch2 * 16 + pos + 1])
                if ch2 == 0:
                    # LN2 residual terms: g1*x1n + (bg1 + b_ffn2); scheduled
                    # here so they run under the FFN matmuls
                    for mt in range(NT):
                        nc.vector.tensor_mul(x1g2[mt], x1n[mt], g1_b)
                        nc.vector.tensor_add(x1g2[mt], x1g2[mt], b2g_b)
                for nb in range(2):
                    w2t = [w2s.tile([P, 512], BF16, name=f"w2_{kc}", tag="w2")
                           for kc in range(16)]
                    for kc in range(16):
                        nc.sync.dma_start(
                            out=w2t[kc],
                            in_=w2[ch2 * 2048 + kc * P: ch2 * 2048 + (kc + 1) * P,
                                   ts(nb, 512)])
                    for mt in range(NT):
                        pt = psU.tile([P, 512], F32, tag="u", name="px2")
                        for kc in range(16):
                            nc.tensor.matmul(
                                pt, hT[kc][:, ts(mt, P)], w2t[kc],
                                start=(kc == 0), stop=(kc == 15))
                        if ch2 == 0:
                            # xf = psum + residual term (g1*x1n + bg1 + b2)
                            nc.vector.scalar_tensor_tensor(
                                out=xf2[mt][:, ts(nb, 512)], in0=pt, scalar=0.0,
                                in1=x1g2[mt][:, ts(nb, 512)],
                                op0=ALU.add, op1=ALU.add)
                        else:
                            nc.vector.tensor_add(
                                xf2[mt][:, ts(nb, 512)], xf2[mt][:, ts(nb, 512)],
                                pt)
            # LN2 + store, pipelined behind the pass-1 accumulation
            for mt in range(NT):
                yo = ftmp.tile([P, D], F32, name="yo", tag="yo")
                ln_natural(ftmp, xf2[mt], yo, "f", g_b=g2_b, bg_b=bg2_b)
                nc.sync.dma_start(out=out[ts(mt, P), :], in_=yo)
    return nc


_CACHE = {}


def _get_nc():
    if "nc" not in _CACHE:
        nc = bacc.Bacc(num_devices=NCORES)
        build(nc)
        _CACHE["nc"] = nc
    return _CACHE["nc"]


def _build_in_maps(inputs):
    src = np.ascontiguousarray(inputs["src"], dtype=np.float32)      # [B,N,D]
    mask = np.asarray(inputs["mask"])                                # [B,N] bool
    keep = (~mask).astype(np.float32)

    import ml_dtypes
    BF = ml_dtypes.bfloat16
    E4 = ml_dtypes.float8_e4m3

    def pack8(w):
        # [1024, C] fp32 -> [512, 2C] fp8, partition kp carries rows
        # kpr*256 + s*128 + kp at free offset s*C + c
        Cw = w.shape[1]
        w4 = w.reshape(KPR, 2, P, Cw).transpose(0, 2, 1, 3).reshape(KPR * P, 2 * Cw)
        return np.ascontiguousarray(np.clip(w4, -240.0, 240.0).astype(E4))

    # stat-column builders: col 2h+s -> s==0: keep (sgn=+1, ofs=0)
    #                                   s==1: 1-keep (sgn=-1, ofs=1)
    sgn = np.tile(np.array([1.0, -1.0], BF), H)[None, :].repeat(P, 0)
    ofs = np.tile(np.array([0.0, 1.0], BF), H)[None, :].repeat(P, 0)

    common = dict(
        wq8=pack8(np.asarray(inputs["wq"], np.float32)),
        wk8=pack8(np.asarray(inputs["wk"], np.float32)),
        wv8=pack8(np.asarray(inputs["wv"], np.float32)),
        wc8=pack8(np.asarray(inputs["w_concat"], np.float32)),
        w1=np.ascontiguousarray(
            (np.asarray(inputs["ln1_g"], np.float32)[:, None]
             * np.asarray(inputs["w_ffn1"], np.float32)).astype(BF)),
        w2=np.ascontiguousarray(np.asarray(inputs["w_ffn2"], np.float32).astype(BF)),
        sgn_row=np.ascontiguousarray(sgn),
        ofs_row=np.ascontiguousarray(ofs),
        b1_col=np.ascontiguousarray(
            (np.asarray(inputs["b_ffn1"], np.float32)
             + np.asarray(inputs["ln1_b"], np.float32)
             @ np.asarray(inputs["w_ffn1"], np.float32)
             ).reshape(C4 // P, P).T),
        b2_row=np.ascontiguousarray(
            (np.asarray(inputs["b_ffn2"], np.float32)
             + np.asarray(inputs["ln1_b"], np.float32)).reshape(1, D)),
        g1_row=np.ascontiguousarray(
            np.asarray(inputs["ln1_g"], np.float32).reshape(1, D)),
        bg1_row=np.ascontiguousarray(
            np.asarray(inputs["ln1_b"], np.float32).reshape(1, D)),
        g2_row=np.ascontiguousarray(
            np.asarray(inputs["ln2_g"], np.float32).reshape(1, D)),
        bg2_row=np.ascontiguousarray(
            np.asarray(inputs["ln2_b"], np.float32).reshape(1, D)),
    )
    bc = np.asarray(inputs["b_concat"], np.float32).reshape(1, D)

    in_maps = []
    for b in range(NCORES):
        m = dict(common)
        m["src"] = np.ascontiguousarray(src[b] + bc)   # fold b_concat into residual
        m["srcT8"] = pack8(np.ascontiguousarray(src[b].T))
        m["keep_row"] = np.ascontiguousarray(keep[b].reshape(1, N))
        m["keep_col"] = np.ascontiguousarray(keep[b].reshape(NT, P).T)
        in_maps.append(m)
    return in_maps


def kernel(**inputs):
    in_maps = _build_in_maps(inputs)

    from concourse.bass_utils import run_bass_kernel_spmd

    nc = _get_nc()
    if not nc.is_finalized():
        nc.finalize()
    res = run_bass_kernel_spmd(nc, in_maps, core_ids=list(range(NCORES)))
    return np.stack([res.results[b]["out"] for b in range(NCORES)], axis=0)


if __name__ == "__main__":
    nc = bacc.Bacc(num_devices=NCORES)
    build(nc)
    print("build OK; instructions:",
          sum(len(bb.instructions) for bb in nc.main_func.blocks))


# revision 26
# speedup vs baseline: 1.0057x; 1.0057x over previous
"""Trainium2 Bass kernel for nn_Attention_58153857187952.

Dense transformer block: QKV -> masked softmax attention (with a global-max
mask bias) -> concat proj -> post-LN residual -> FFN(gelu) -> post-LN.

Sharding: batch data-parallel, 1 batch element per core (B=8, 8 cores).

Math: the reference computes
    attn = softmax(qk + (1-m)*(-gmax)) * m,   gmax = max(qk) over ALL b,h,i,j
Softmax rows decompose:
    out_ij = p_ij * keep_j / (D1_i + e^{-gmax} * D2_i),  p = exp(qk)
with D1 = sum_keep p, D2 = sum_masked p.  Scores are bounded (|qk| < ~8) so
exp needs no row-max subtraction.  e^{-gmax} enters only as a ~0.3%
denominator correction, so a per-core sampled max is numerically
indistinguishable from the global max -> no collective needed.  The exp is
computed shifted (p' = exp(qk*SCALE - C)) so probabilities fit fp8 e4m3;
the shift cancels exactly in the softmax ratio (eg is rescaled by e^-C).

Perf structure (final):
  * QKV + concat projections and the PV matmul run in fp8 (e4m3)
    DoubleRow mode (2 k-rows per PE pass); weights, src^T and the packed
    V/p operands are laid out [128, 2, C] (host-side packing for weights
    and src^T, on-device for p/V/attnT).
  * Score matmuls row-packed per head pair (K=64 at array rows 0-63 and
    64-127) run concurrently; PV chain matmuls are interleaved with the
    exp calls so the PE never idles a full HAM window.
  * V carries [keep, 1-keep] stat columns (66-wide lhsT) so D1/D2 fall
    out of the PV matmul with zero extra streaming; per-head-pair
    denominator/scale chains are pipelined one pair behind the attention
    loop with broadcast reads straight from the stats scratch.
  * exp/gelu/psum-copies operate on [128,1024] tiles; layernorm uses
    bn_stats/bn_aggr; LN1 gains are folded into w1/b1 host-side so LN1
    emits bf16 directly; b_concat is folded into src, and bg1+b_ffn2 into
    the FFN2 accumulation; the LN2 residual term is precomputed under the
    FFN matmuls.
  * x1^T comes from full-column DMA xbar transposes; the FFN runs bf16 in
    two half-C4 passes so LN2 + the output DMA pipeline behind FFN2.
"""

import os
import sys

import numpy as np

sys.path.insert(0, "/opt/trn_rl_repo")

from contextlib import ExitStack

import concourse.bass as bass
import concourse.tile as tile
from concourse import bacc
from concourse import mybir
from concourse.bass import ts

B, N, D, H = 8, 1024, 1024, 16
HD = D // H
SCALE = HD ** -0.5
EPS = 1e-5
P = 128
NT = N // P          # 8 token tiles
DT = D // P          # 8 feature tiles
KPR = D // 256       # 4 packed k-pair groups
C4 = 4 * D           # 4096
NCORES = 8
HW = HD + 2          # 66: head dims + [keep, kinv] stat columns
EXPC = 3.0           # exp shift: p' = exp(qk*SCALE - C) keeps fp8 < 240
EGC = float(np.exp(-EXPC))

F32 = mybir.dt.float32
BF16 = mybir.dt.bfloat16
F8 = mybir.dt.float8e4
AX = mybir.AxisListType.X
ALU = mybir.AluOpType
ACTF = mybir.ActivationFunctionType
DR = mybir.MatmulPerfMode.DoubleRow


def _bc(ap, parts):
    """Partition-broadcast a [1, ...] DRAM AP across `parts` partitions."""
    return bass.AP(tensor=ap.tensor, offset=ap.offset, ap=[[0, parts]] + list(ap.ap[1:]))


def build(nc):
    # ---------------- I/O ----------------
    src = nc.declare_dram_parameter("src", [N, D], F32, isOutput=False)  # src + b_concat
    srcT8 = nc.declare_dram_parameter("srcT8", [KPR * P, 2 * N], F8, isOutput=False)
    wq8 = nc.declare_dram_parameter("wq8", [KPR * P, 2 * D], F8, isOutput=False)
    wk8 = nc.declare_dram_parameter("wk8", [KPR * P, 2 * D], F8, isOutput=False)
    wv8 = nc.declare_dram_parameter("wv8", [KPR * P, 2 * D], F8, isOutput=False)
    wc8 = nc.declare_dram_parameter("wc8", [KPR * P, 2 * D], F8, isOutput=False)
    w1 = nc.declare_dram_parameter("w1", [D, C4], BF16, isOutput=False)
    w2 = nc.declare_dram_parameter("w2", [C4, D], BF16, isOutput=False)
    keep_row = nc.declare_dram_parameter("keep_row", [1, N], F32, isOutput=False)
    keep_col = nc.declare_dram_parameter("keep_col", [P, NT], F32, isOutput=False)
    sgn_row = nc.declare_dram_parameter("sgn_row", [P, 2 * H], BF16, isOutput=False)
    ofs_row = nc.declare_dram_parameter("ofs_row", [P, 2 * H], BF16, isOutput=False)
    b1_col = nc.declare_dram_parameter("b1_col", [P, C4 // P], F32, isOutput=False)
    b2_row = nc.declare_dram_parameter("b2_row", [1, D], F32, isOutput=False)
    g1_row = nc.declare_dram_parameter("g1_row", [1, D], F32, isOutput=False)
    bg1_row = nc.declare_dram_parameter("bg1_row", [1, D], F32, isOutput=False)
    g2_row = nc.declare_dram_parameter("g2_row", [1, D], F32, isOutput=False)
    bg2_row = nc.declare_dram_parameter("bg2_row", [1, D], F32, isOutput=False)
    out = nc.declare_dram_parameter("out", [N, D], F32, isOutput=True)

    # internal DRAM scratch
    dstat_dram = nc.dram_tensor("dstat_dram", [2 * H, N], F32)  # rows h: D1, 16+h: D2
    gcol_dram = nc.dram_tensor("gcol_dram", [P, 1], F32)
    eg_dram = nc.dram_tensor("eg_dram", [1, 1], F32)
    s16_dram = nc.dram_tensor("s16_dram", [H, N], F32)
    x1bd_dram = nc.dram_tensor("x1bd_dram", [N, D], BF16)

    def ln_natural(pool, xin, yout, tagp, g_b=None, bg_b=None):
        """Layernorm along free dim of a [P, D] tile into caller tile yout.

        Without g_b/bg_b, writes the normalized value directly (gains folded
        into downstream weights host-side)."""
        stats = pool.tile([P, 2, 6], F32, name=tagp + "st", tag=tagp + "st")
        xr = xin.rearrange("p (a b) -> p a b", b=512)
        nc.vector.bn_stats(out=stats[:, 0, :], in_=xr[:, 0, :])
        nc.vector.bn_stats(out=stats[:, 1, :], in_=xr[:, 1, :])
        mv = pool.tile([P, 2], F32, name=tagp + "mv", tag=tagp + "mv")
        nc.vector.bn_aggr(out=mv, in_=stats)
        std = pool.tile([P, 1], F32, name=tagp + "s5", tag=tagp + "s5")
        nc.scalar.activation(std, mv[:, 1:2], ACTF.Sqrt, bias=eps_t)
        rstd = pool.tile([P, 1], F32, name=tagp + "s6", tag=tagp + "s6")
        nc.vector.reciprocal(rstd, std)
        nc.vector.tensor_scalar(out=yout, in0=xin, scalar1=mv[:, 0:1],
                                scalar2=rstd, op0=ALU.subtract, op1=ALU.mult)
        if g_b is not None:
            nc.vector.tensor_mul(yout, yout, g_b)
            nc.vector.tensor_add(yout, yout, bg_b)

    with ExitStack() as ctx:
        tc = ctx.enter_context(tile.TileContext(nc))
        sing = ctx.enter_context(tc.tile_pool(name="sing", bufs=1))
        psS = ctx.enter_context(tc.tile_pool(name="psS", bufs=3, space="PSUM"))
        psU = ctx.enter_context(tc.tile_pool(name="psU", bufs=2, space="PSUM"))
        x1T_pool = ctx.enter_context(tc.tile_pool(name="x1T", bufs=1))
        x1_pool = ctx.enter_context(tc.tile_pool(name="x1p", bufs=1))

        keepc = sing.tile([P, NT], F32)
        nc.sync.dma_start(out=keepc, in_=keep_col[:])
        b1c = sing.tile([P, C4 // P], F32)
        nc.sync.dma_start(out=b1c, in_=b1_col[:])
        sgn = sing.tile([P, 2 * H], BF16)
        nc.sync.dma_start(out=sgn, in_=sgn_row[:])
        ofs = sing.tile([P, 2 * H], BF16)
        nc.sync.dma_start(out=ofs, in_=ofs_row[:])
        pmax_slots = sing.tile([P, 4], F32)
        eps_t = sing.tile([P, 1], F32)
        nc.vector.memset(eps_t, EPS)
        negc_t = sing.tile([P, 1], F32)
        nc.vector.memset(negc_t, -EXPC)

        x1T = [x1T_pool.tile([P, N], BF16, name=f"x1T{t}") for t in range(DT)]
        x1n = [x1_pool.tile([P, D], BF16, name=f"x1n{t}") for t in range(NT)]
        x1g2 = [x1_pool.tile([P, D], BF16, name=f"x1g2_{t}") for t in range(NT)]

        with tc.tile_pool(name="attp", bufs=1) as attp, \
             tc.tile_pool(name="wc8p", bufs=1) as wc8p:
          attnT = [attp.tile([P, N], BF16, name=f"attnT{t}") for t in range(DT)]
          attnT8 = [attp.tile([P, 2, N], F8, name=f"attnT8_{t}") for t in range(KPR)]
          wc8t = [wc8p.tile([P, 2, D], F8, name=f"wc8_{k}") for k in range(KPR)]
          for k in range(KPR):
              nc.sync.dma_start(out=wc8t[k], in_=wc8[ts(k, P), :])

          with tc.tile_pool(name="qkvp", bufs=1) as qkvp:
            QT = [qkvp.tile([P, N], BF16, name=f"qt{t}") for t in range(DT)]
            KT = [qkvp.tile([P, N], BF16, name=f"kt{t}") for t in range(DT)]
            VA8 = [qkvp.tile([P, 2, H * HW], F8, name=f"va{t}")
                   for t in range(NT // 2)]

            # ---------- phase 0/1: load packed operands, QKV projections ----
            with tc.tile_pool(name="w8p", bufs=1) as w8p:
                st8 = [w8p.tile([P, 2, N], F8, name=f"st8_{k}") for k in range(KPR)]
                wq8t = [w8p.tile([P, 2, D], F8, name=f"wq8_{k}") for k in range(KPR)]
                wk8t = [w8p.tile([P, 2, D], F8, name=f"wk8_{k}") for k in range(KPR)]
                wv8t = [w8p.tile([P, 2, D], F8, name=f"wv8_{k}") for k in range(KPR)]
                for k in range(KPR):
                    nc.sync.dma_start(out=st8[k], in_=srcT8[ts(k, P), :])
                    nc.sync.dma_start(out=wq8t[k], in_=wq8[ts(k, P), :])

                # Q^T then K^T: [dq, i] tiles; two heads per tile t
                def qk_proj(w8, dstT, t):
                    pt = psS.tile([P, 1024], F32, tag="mm", name="pqk")
                    for nb in range(2):
                        for k in range(KPR):
                            nc.tensor.matmul(
                                pt[:, ts(nb, 512)],
                                w8[k][:, :, ts(t, P)],
                                st8[k][:, :, ts(nb, 512)],
                                start=(k == 0), stop=(k == KPR - 1),
                                perf_mode=DR)
                    nc.scalar.copy(out=dstT[t], in_=pt)

                for t in range(DT):
                    qk_proj(wq8t, QT, t)
                for k in range(KPR):
                    nc.sync.dma_start(out=wk8t[k], in_=wk8[ts(k, P), :])
                for t in range(DT):
                    qk_proj(wk8t, KT, t)
                for k in range(KPR):
                    nc.sync.dma_start(out=wv8t[k], in_=wv8[ts(k, P), :])
                # V natural [token, dv], keep-zeroed rows + stat columns,
                # packed 2 token-tiles deep for DoubleRow PV
                for it in range(NT):
                    dst = VA8[it // 2][:, it % 2, :].rearrange(
                        "p (h c) -> p h c", c=HW)
                    for nb in range(2):
                        vps = psU.tile([P, 512], F32, tag="u", name="pv")
                        for k in range(KPR):
                            nc.tensor.matmul(
                                vps,
                                st8[k][:, :, ts(it, P)],
                                wv8t[k][:, :, ts(nb, 512)],
                                start=(k == 0), stop=(k == KPR - 1),
                                perf_mode=DR)
                        nc.vector.tensor_scalar(
                            out=dst[:, nb * 8:(nb + 1) * 8, 0:HD],
                            in0=vps.rearrange("p (h c) -> p h c", c=HD),
                            scalar1=keepc[:, it:it + 1], scalar2=None,
                            op0=ALU.mult)
                    # stat cols [keep, kinv] via sgn*keep + ofs
                    kk = qkvp.tile([P, 2 * H], BF16, name="kkt", tag="kkt")
                    nc.vector.tensor_scalar(
                        out=kk, in0=sgn, scalar1=keepc[:, it:it + 1],
                        scalar2=None, op0=ALU.mult)
                    nc.vector.tensor_tensor(
                        out=dst[:, :, HD:HW],
                        in0=kk.rearrange("p (h c) -> p h c", c=2),
                        in1=ofs.rearrange("p (h c) -> p h c", c=2),
                        op=ALU.add)

            # ---------- phase 2+3: attention with pipelined stat chains ----
            with tc.tile_pool(name="ptp", bufs=12) as ptp, \
                 tc.tile_pool(name="st2p", bufs=3) as st2p, \
                 tc.tile_pool(name="ep", bufs=1) as ep, \
                 tc.tile_pool(name="srp", bufs=2) as srp:

                keep128 = ep.tile([P, N], F32)
                nc.sync.dma_start(out=keep128, in_=_bc(keep_row[:], P))
                egp = ep.tile([P, 1], F32)

                def eg_chain():
                    # local sampled max of exp scores -> e^{-gmax} broadcast
                    gmax128 = ep.tile([P, 1], F32)
                    nc.vector.reduce_max(out=gmax128, in_=pmax_slots, axis=AX)
                    nc.sync.dma_start(out=gcol_dram[:], in_=gmax128)
                    grow = ep.tile([1, P], F32)
                    nc.sync.dma_start(out=grow, in_=bass.AP(
                        tensor=gcol_dram[:].tensor, offset=0, ap=[[0, 1], [1, P]]))
                    gmax1 = ep.tile([1, 1], F32)
                    nc.vector.reduce_max(out=gmax1, in_=grow, axis=AX)
                    eg1 = ep.tile([1, 1], F32)
                    nc.vector.reciprocal(eg1, gmax1)
                    nc.vector.tensor_scalar(out=eg1, in0=eg1, scalar1=EGC,
                                            scalar2=None, op0=ALU.mult)
                    nc.sync.dma_start(out=eg_dram[:], in_=eg1)
                    nc.sync.dma_start(out=egp, in_=_bc(eg_dram[:], P))

                def stat_chain(t):
                    # scale s(h,i) = keep_i / (D1 + eg*D2) for heads 2t, 2t+1,
                    # with D1/D2 broadcast-read straight from dstat rows
                    da = dstat_dram[:].tensor
                    b1 = srp.tile([P, N], F32, name="sb1", tag="sb1")
                    nc.scalar.dma_start(out=b1[0:HD, :], in_=bass.AP(
                        tensor=da, offset=2 * t * N, ap=[[0, HD], [1, N]]))
                    nc.scalar.dma_start(out=b1[HD:P, :], in_=bass.AP(
                        tensor=da, offset=(2 * t + 1) * N, ap=[[0, HD], [1, N]]))
                    b2t = srp.tile([P, N], F32, name="sb2", tag="sb2")
                    nc.scalar.dma_start(out=b2t[0:HD, :], in_=bass.AP(
                        tensor=da, offset=(H + 2 * t) * N, ap=[[0, HD], [1, N]]))
                    nc.scalar.dma_start(out=b2t[HD:P, :], in_=bass.AP(
                        tensor=da, offset=(H + 2 * t + 1) * N, ap=[[0, HD], [1, N]]))
                    ss = srp.tile([P, N], F32, name="ss", tag="ss")
                    nc.vector.tensor_scalar(out=ss, in0=b2t, scalar1=egp,
                                            scalar2=None, op0=ALU.mult)
                    nc.vector.tensor_add(ss, ss, b1)
                    nc.vector.reciprocal(ss, ss)
                    nc.vector.tensor_mul(ss, ss, keep128)
                    nc.vector.tensor_tensor(
                        out=attnT8[t // 2][:, t % 2, :],
                        in0=attnT[t], in1=ss, op=ALU.mult)

                for t in range(DT):  # head pair (2t, 2t+1)
                    for ih in range(2):
                        # PV accumulators for both heads, fed as exps land so
                        # the PE never idles a full HAM window
                        uts = [psU.tile([P, 512], F32, tag="u", name="ut")
                               for _ in range(2)]

                        def pv_step(jg, PA, PB):
                            for hh, Ptile in ((0, PA), (1, PB)):
                                p8 = Ptile.rearrange("p (s i) -> p s i", i=512)
                                nc.tensor.matmul(
                                    uts[hh][0:HW, :],
                                    VA8[jg][:, :, ts(2 * t + hh, HW)],
                                    p8,
                                    start=(jg == 0), stop=(jg == 3),
                                    perf_mode=DR)

                        ptiles = []
                        for jg in range(4):
                            SA = psS.tile([P, 1024], F32, tag="mm", name="sa")
                            SB = psS.tile([P, 1024], F32, tag="mm", name="sb")
                            for jj in range(2):
                                jt = 2 * jg + jj
                                nc.tensor.matmul(
                                    SA[:, ts(jj, 512)],
                                    KT[t][0:HD, ts(jt, P)],
                                    QT[t][0:HD, ts(ih, 512)],
                                    start=True, stop=True)
                                nc.tensor.matmul(
                                    SB[:, ts(jj, 512)],
                                    KT[t][HD:P, ts(jt, P)],
                                    QT[t][HD:P, ts(ih, 512)],
                                    start=True, stop=True)
                            PA = ptp.tile([P, 1024], F8, tag="pt", name="pa")
                            PB = ptp.tile([P, 1024], F8, tag="pt", name="pb")
                            nc.scalar.activation(PA, SA, ACTF.Exp, scale=SCALE,
                                                 bias=negc_t)
                            nc.scalar.activation(PB, SB, ACTF.Exp, scale=SCALE,
                                                 bias=negc_t)
                            if t < 2 and ih == 0 and jg == 1:
                                nc.vector.reduce_max(
                                    out=pmax_slots[:, 2 * t:2 * t + 1],
                                    in_=PA, axis=AX)
                                nc.vector.reduce_max(
                                    out=pmax_slots[:, 2 * t + 1:2 * t + 2],
                                    in_=PB, axis=AX)
                            ptiles.append((PA, PB))
                            if jg >= 1:
                                pv_step(jg - 1, *ptiles[jg - 1])
                        pv_step(3, *ptiles[3])
                        for hh in range(2):
                            h = 2 * t + hh
                            ut = uts[hh]
                            nc.vector.tensor_copy(
                                out=attnT[t][hh * HD:(hh + 1) * HD, ts(ih, 512)],
                                in_=ut[0:HD, :])
                            st2 = st2p.tile([2, 512], F32, name="st2", tag="st2")
                            nc.vector.tensor_copy(out=st2, in_=ut[HD:HW, :])
                            nc.sync.dma_start(
                                out=bass.AP(tensor=dstat_dram[:].tensor,
                                            offset=h * N + ih * 512,
                                            ap=[[H * N, 2], [1, 512]]),
                                in_=st2)
                    if t == 1:
                        eg_chain()
                    if t >= 1:
                        stat_chain(t - 1)
                stat_chain(DT - 1)

          # ---------- phase 4: concat proj + residual + LN1 + x1T ----------
          with tc.tile_pool(name="ctmp", bufs=3) as ctmp:
              for mt in range(NT):
                  x0ps = psS.tile([P, 1024], F32, tag="mm", name="x0ps")
                  for nb in range(2):
                      for k in range(KPR):
                          nc.tensor.matmul(
                              x0ps[:, ts(nb, 512)],
                              attnT8[k][:, :, ts(mt, P)],
                              wc8t[k][:, :, ts(nb, 512)],
                              start=(k == 0), stop=(k == KPR - 1),
                              perf_mode=DR)
                  sre = ctmp.tile([P, D], F32, name="sr", tag="sr")
                  nc.scalar.dma_start(out=sre, in_=src[ts(mt, P), :])
                  x0 = ctmp.tile([P, D], F32, name="x0", tag="x0")
                  nc.vector.scalar_tensor_tensor(
                      out=x0, in0=x0ps, scalar=0.0, in1=sre,
                      op0=ALU.add, op1=ALU.add)
                  # g1/bg1 folded into w1/b1 and the residual term below
                  ln_natural(ctmp, x0, x1n[mt], "c")
                  nc.sync.dma_start(out=x1bd_dram[ts(mt, P), :], in_=x1n[mt])
              for kt in range(DT):
                  nc.scalar.dma_start_transpose(
                      out=x1T[kt], in_=x1bd_dram[:, ts(kt, P)])

        # ---------- phase 5: FFN (two half-C4 passes) + pipelined LN2 ----
        with tc.tile_pool(name="ffn", bufs=1) as ffp, \
             tc.tile_pool(name="w1s", bufs=16) as w1s, \
             tc.tile_pool(name="w2s", bufs=24) as w2s, \
             tc.tile_pool(name="ftmp", bufs=3) as ftmp:
            g1_b = ffp.tile([P, D], F32)
            nc.sync.dma_start(out=g1_b, in_=_bc(g1_row[:], P))
            b2g_b = ffp.tile([P, D], F32)
            nc.sync.dma_start(out=b2g_b, in_=_bc(b2_row[:], P))
            g2_b = ffp.tile([P, D], F32)
            nc.sync.dma_start(out=g2_b, in_=_bc(g2_row[:], P))
            bg2_b = ffp.tile([P, D], F32)
            nc.sync.dma_start(out=bg2_b, in_=_bc(bg2_row[:], P))
            hT = [ffp.tile([P, N], BF16, name=f"hT{t}") for t in range(16)]
            xf2 = [ffp.tile([P, D], F32, name=f"xf{t}") for t in range(NT)]
            for ch2 in range(2):
                for chh in range(4):
                    w1t = [w1s.tile([P, 512], BF16, name=f"w1_{kt}", tag="w1")
                           for kt in range(DT)]
                    for kt in range(DT):
                        nc.sync.dma_start(
                            out=w1t[kt],
                            in_=w1[ts(kt, P),
                                   ch2 * 2048 + chh * 512:
                                   ch2 * 2048 + (chh + 1) * 512])
                    for mc in range(4):
                        pos = chh * 4 + mc
                        hps = psS.tile([P, 1024], F32, tag="mm", name="hps")
                        for nb in range(2):
                            for kt in range(DT):
                                nc.tensor.matmul(
                                    hps[:, ts(nb, 512)],
                                    w1t[kt][:, ts(mc, P)],
                                    x1T[kt][:, ts(nb, 512)],
                                    start=(kt == 0), stop=(kt == DT - 1))
                        nc.scalar.activation(
                            hT[pos], hps, ACTF.Gelu,
                            bias=b1c[:, ch2 * 16 + pos: ch2 * 16 + pos + 1])
                if ch2 == 0:
                    # LN2 residual terms: g1*x1n + (bg1 + b_ffn2); scheduled
                    # here so they run under the FFN matmuls
                    for mt in range(NT):
                        nc.vector.tensor_mul(x1g2[mt], x1n[mt], g1_b)
                        nc.vector.tensor_add(x1g2[mt], x1g2[mt], b2g_b)
                for nb in range(2):
                    w2t = [w2s.tile([P, 512], BF16, name=f"w2_{kc}", tag="w2")
                           for kc in range(16)]
                    for kc in range(16):
                        nc.sync.dma_start(
                            out=w2t[kc],
                            in_=w2[ch2 * 2048 + kc * P: ch2 * 2048 + (kc + 1) * P,
                                   ts(nb, 512)])
                    for mt in range(NT):
                        pt = psU.tile([P, 512], F32, tag="u", name="px2")
                        for kc in range(16):
                            nc.tensor.matmul(
                                pt, hT[kc][:, ts(mt, P)], w2t[kc],
                                start=(kc == 0), stop=(kc == 15))
                        if ch2 == 0:
                            # xf = psum + residual term (g1*x1n + bg1 + b2)
                            nc.vector.scalar_tensor_tensor(
                                out=xf2[mt][:, ts(nb, 512)], in0=pt, scalar=0.0,
                                in1=x1g2[mt][:, ts(nb, 512)],
                                op0=ALU.add, op1=ALU.add)
                        else:
                            nc.vector.tensor_add(
                                xf2[mt][:, ts(nb, 512)], xf2[mt][:, ts(nb, 512)],
                                pt)
            # LN2 + store, pipelined behind the pass-1 accumulation
            for mt in range(NT):
                yo = ftmp.tile([P, D], F32, name="yo", tag="yo")
                ln_natural(ftmp, xf2[mt], yo, "f", g_b=g2_b, bg_b=bg2_b)
                nc.sync.dma_start(out=out[ts(mt, P), :], in_=yo)
    return nc


_CACHE = {}


def _get_nc():
    if "nc" not in _CACHE:
        nc = bacc.Bacc(num_devices=NCORES)
        build(nc)
        _CACHE["nc"] = nc
    return _CACHE["nc"]


def _build_in_maps(inputs):
    src = np.ascontiguousarray(inputs["src"], dtype=np.float32)      # [B,N,D]
    mask = np.asarray(inputs["mask"])                                # [B,N] bool
    keep = (~mask).astype(np.float32)

    import ml_dtypes
    BF = ml_dtypes.bfloat16
    E4 = ml_dtypes.float8_e4m3

    def pack8(w):
        # [1024, C] fp32 -> [512, 2C] fp8, partition kp carries rows
        # kpr*256 + s*128 + kp at free offset s*C + c
        Cw = w.shape[1]
        w4 = w.reshape(KPR, 2, P, Cw).transpose(0, 2, 1, 3).reshape(KPR * P, 2 * Cw)
        return np.ascontiguousarray(np.clip(w4, -240.0, 240.0).astype(E4))

    # stat-column builders: col 2h+s -> s==0: keep (sgn=+1, ofs=0)
    #                                   s==1: 1-keep (sgn=-1, ofs=1)
    sgn = np.tile(np.array([1.0, -1.0], BF), H)[None, :].repeat(P, 0)
    ofs = np.tile(np.array([0.0, 1.0], BF), H)[None, :].repeat(P, 0)

    common = dict(
        wq8=pack8(np.asarray(inputs["wq"], np.float32)),
        wk8=pack8(np.asarray(inputs["wk"], np.float32)),
        wv8=pack8(np.asarray(inputs["wv"], np.float32)),
        wc8=pack8(np.asarray(inputs["w_concat"], np.float32)),
        w1=np.ascontiguousarray(
            (np.asarray(inputs["ln1_g"], np.float32)[:, None]
             * np.asarray(inputs["w_ffn1"], np.float32)).astype(BF)),
        w2=np.ascontiguousarray(np.asarray(inputs["w_ffn2"], np.float32).astype(BF)),
        sgn_row=np.ascontiguousarray(sgn),
        ofs_row=np.ascontiguousarray(ofs),
        b1_col=np.ascontiguousarray(
            (np.asarray(inputs["b_ffn1"], np.float32)
             + np.asarray(inputs["ln1_b"], np.float32)
             @ np.asarray(inputs["w_ffn1"], np.float32)
             ).reshape(C4 // P, P).T),
        b2_row=np.ascontiguousarray(
            (np.asarray(inputs["b_ffn2"], np.float32)
             + np.asarray(inputs["ln1_b"], np.float32)).reshape(1, D)),
        g1_row=np.ascontiguousarray(
            np.asarray(inputs["ln1_g"], np.float32).reshape(1, D)),
        bg1_row=np.ascontiguousarray(
            np.asarray(inputs["ln1_b"], np.float32).reshape(1, D)),
        g2_row=np.ascontiguousarray(
            np.asarray(inputs["ln2_g"], np.float32).reshape(1, D)),
        bg2_row=np.ascontiguousarray(
            np.asarray(inputs["ln2_b"], np.float32).reshape(1, D)),
    )
    bc = np.asarray(inputs["b_concat"], np.float32).reshape(1, D)

    in_maps = []
    for b in range(NCORES):
        m = dict(common)
        m["src"] = np.ascontiguousarray(src[b] + bc)   # fold b_concat into residual
        m["srcT8"] = pack8(np.ascontiguousarray(src[b].T))
        m["keep_row"] = np.ascontiguousarray(keep[b].reshape(1, N))
        m["keep_col"] = np.ascontiguousarray(keep[b].reshape(NT, P).T)
        in_maps.append(m)
    return in_maps


def kernel(**inputs):
    in_maps = _build_in_maps(inputs)

    from concourse.bass_utils import run_bass_kernel_spmd

    nc = _get_nc()
    if not nc.is_finalized():
        nc.finalize()
    res = run_bass_kernel_spmd(nc, in_maps, core_ids=list(range(NCORES)))
    return np.stack([res.results[b]["out"] for b in range(NCORES)], axis=0)


if __name__ == "__main__":
    nc = bacc.Bacc(num_devices=NCORES)
    build(nc)
    print("build OK; instructions:",
          sum(len(bb.instructions) for bb in nc.main_func.blocks))
